# revision 27
# baseline (speedup 1.0000x reference)
"""Trainium2 Bass kernel for nn_EventTemplateBank (batched 1-D template-bank conv).

Math: score[b,t,e] = sum_{f,l} delayed[e,f,l] * x[b, t+40-l, f] / (L*F),
with delayed = delay-shifted templates (zero fill) and x zero-padded.

Device formulation (per core, data-parallel over batch):
  Feature-packed Toeplitz: contraction partitions hold (feature, tap-chunk)
  pairs, K = 6*21 = 126, so one matmul contracts all 6 features over a
  21-tap window. Each rhs column covers Q=24 output positions; the 103-tap
  span (24+79) is accumulated over NCH=5 chunks. Outputs (d in [0,24),
  e in [0,16)) form NM=3 M-tiles of 128.
    X[(f,c), j, col=n] = x[b, 24n + 21j + c - 39, f]      (fp8 e3m4, host im2col)
    W[(f,c), j, m, (dd,e)] = delayed[e, f, 8m+dd+79-21j-c] / 480   (bf16)
    ps[m][(dd,e), n] += sum_j W[:, j, m].T @ X[:, j, n-block]
  163,920 column-passes/core = 68.3 us PE floor at 2.4 GHz.
  X moves as fp8e3 (e3m4: 4 mantissa bits; host-sim rel err 1.38e-2 < 2e-2)
  against bf16 stationary W (mixed-dtype matmul, 1 cycle/row), halving X DMA:
  6.9 MB X + 8.4 MB out + 0.5 MB W ~= 15.8 MB/core ~= 44 us at 360 GB/s --
  DMA is no longer a co-bottleneck with the PE.
  A warmup chain of dummy matmuls on a memset tile runs during the DMA head
  so the PE p-state ramp (~3 us at 1.2 GHz) completes before real work.
  PSUM f32 -> bf16 evac; host upcasts and re-permutes (t = 24n + 8m + dd).
"""

import numpy as np
import ml_dtypes

import concourse.mybir as mybir
from concourse import bacc
from concourse.bass_utils import run_bass_kernel_spmd
from concourse.tile import TileContext

BF16 = ml_dtypes.bfloat16
FP8 = ml_dtypes.float8_e3m4

# Problem shapes (hardcoded per contract)
B, S, F = 64, 32768, 6
E, L = 16, 80
MAX_DELAY = 10

NCORES = 8
BPC = B // NCORES          # batches per core
Q = 24                     # output positions per rhs column
C = 21                     # taps per feature per chunk
NCH = 5                    # accumulation chunks (cover 24+79 = 103 <= 105 taps)
KP = F * C                 # 126 contraction partitions
NM = 3                     # M-tiles: (dd in [0,8)) x (e in [0,16)) per tile
PADL = 39                  # chunk sample index = 24n + 21j + c - 39
NCOLB = (S + Q - 1) // Q   # 1366 columns per batch
CTOT = BPC * NCOLB         # 10928 columns per core
# Lead blocks run j-outer, gated on per-j W pieces; block 0 (128 cols) rides
# in ONE fused DMA together with W(j0,j1) so a single DMA-latency chain
# unlocks the first six matmul groups. 512-col steady state; shrinking tail
# blocks keep the final evac+output-DMA drain chain short.
BLOCKS = [64, 320] + [512] * 19 + [432, 368, 16]
assert sum(BLOCKS) == CTOT
NBLK = len(BLOCKS)
BOFF = [sum(BLOCKS[:i]) for i in range(NBLK)]
N_JOUTER = 2               # leading blocks run j-outer (gate on per-j W pieces)
NJ_HEAD = 2                # W(j0,j1) rides the fused head DMA
HEAD_WB = NJ_HEAD * NM * 128 * 2            # W bytes in the fused head DMA
HEAD_XB = NCH * BLOCKS[0]                   # X block-0 bytes in the head DMA
NWARM = 33                 # warmup fills the whole pre-data window (~3.5 us)

LAST_RESULT = None         # BassKernelResults of the most recent run (for profiling)


def _build_weights(templates: np.ndarray, onset_delays: np.ndarray) -> np.ndarray:
    """W[(f,c), j, m, 16dd+e] = delayed[e, f, 8m+dd+79-21j-c] / (L*F)."""
    d = np.round(np.clip(onset_delays, -MAX_DELAY, MAX_DELAY)).astype(np.int64)
    idx = np.arange(L)
    src = idx[None, None, :] - d[:, :, None]                 # (E,F,L)
    valid = (src >= 0) & (src < L)
    delayed = np.take_along_axis(templates, np.clip(src, 0, L - 1), axis=2)
    delayed = np.where(valid, delayed, 0.0).astype(np.float32) / float(L * F)

    f_i = np.arange(F)[:, None, None, None, None]
    c_i = np.arange(C)[None, :, None, None, None]
    j_i = np.arange(NCH)[None, None, :, None, None]
    dd_i = np.arange(8)[None, None, None, :, None]
    m_i = np.arange(NM)[None, None, None, None, :]
    l = (8 * m_i + dd_i) + 79 - 21 * j_i - c_i               # (F,C,NCH,8,NM)
    ok = (l >= 0) & (l < L)
    g = delayed[:, f_i, np.clip(l, 0, L - 1)]                # (E,F,C,NCH,8,NM)
    g = np.where(ok[None], g, 0.0)
    # -> [(f,c), j, m, dd, e]
    W = g.transpose(1, 2, 3, 5, 4, 0).reshape(KP, NCH, NM, 8 * E)
    return np.ascontiguousarray(W).astype(BF16)


def _build_xsc(x: np.ndarray) -> np.ndarray:
    """Xsc[core, (f,c), :] = block-major concat of [NCH, n_blk] chunk rows:
    chunk j of column col = 1366*b_local + n reads x[b, 24n + 21j + c - 39, f].
    Quantized to fp8 e3m4 once on the raw samples (all replicas identical)."""
    need = Q * (NCOLB - 1) + 21 * (NCH - 1) + C
    xpad = np.zeros((B, PADL + need, F), dtype=np.float32)
    xpad[:, PADL:PADL + S, :] = x
    xpad8 = xpad.astype(FP8)
    sb, st, sf = xpad8.strides
    # V[b, (f,c), j, n] = xpad8[b, 24n + 21j + c, f]
    V = np.lib.stride_tricks.as_strided(
        xpad8, shape=(B, F, C, NCH, NCOLB), strides=(sb, sf, st, 21 * st, Q * st)
    )
    V8 = np.ascontiguousarray(V).reshape(B, KP, NCH, NCOLB)
    Xc = np.empty((NCORES, KP, NCH, CTOT), dtype=FP8)
    for b in range(B):
        core, i = divmod(b, BPC)
        Xc[core, :, :, i * NCOLB:(i + 1) * NCOLB] = V8[b]
    out = np.empty((NCORES, KP, NCH * CTOT), dtype=FP8)
    for off, n in zip(BOFF, BLOCKS):
        out[:, :, NCH * off:NCH * (off + n)] = (
            Xc[:, :, :, off:off + n].reshape(NCORES, KP, NCH * n)
        )
    return np.ascontiguousarray(out)


def _build_program():
    f32 = mybir.dt.float32
    bf16 = mybir.dt.bfloat16
    fp8 = mybir.dt.float8e3
    nc = bacc.Bacc("TRN2", target_bir_lowering=False, debug=False)
    xsc = nc.dram_tensor("xsc", [KP, NCH * CTOT], fp8, kind="ExternalInput")
    # Fused head: W(j0,j1) raw bf16 bytes followed by X block-0, one DMA.
    hd = nc.dram_tensor("hd", [KP, HEAD_WB + HEAD_XB], fp8, kind="ExternalInput")
    w = nc.dram_tensor("w", [KP, NCH, NM, 128], bf16, kind="ExternalInput")
    osc = nc.dram_tensor("osc", [128, NM * CTOT], bf16, kind="ExternalOutput")

    with TileContext(nc) as tc:
        with (
            tc.tile_pool(name="wp", bufs=1) as wp,
            tc.tile_pool(name="xp", bufs=6) as xp,
            tc.tile_pool(name="pp", bufs=7, space="PSUM") as pp,
            tc.tile_pool(name="wu", bufs=1) as wup,
            tc.tile_pool(name="wq", bufs=1, space="PSUM") as wqp,
            tc.tile_pool(name="op", bufs=6) as op,
        ):
            # --- PE warmup: dummy matmuls on a zeroed tile ramp the tensor
            # engine's p-state to full clock while the first DMAs land.
            wlhs = wup.tile([128, 128], bf16)
            nc.gpsimd.memset(wlhs, 0)
            wps = wqp.tile([128, 128], f32)
            for _ in range(NWARM):
                nc.tensor.matmul(
                    wps, wlhs, wlhs, start=True, stop=True, skip_group_check=True
                )

            ht = wp.tile([KP, HEAD_WB + HEAD_XB], fp8)   # fused W(j0,j1) + X0
            htw = ht[:, :HEAD_WB].bitcast(bf16)          # [(f,c), 768 bf16]
            wt = wp.tile([KP, (NCH - NJ_HEAD) * NM * 128], bf16)  # W(j2..j4)
            wr = w.rearrange("k j m n -> k (j m n)")
            xtiles = {}

            def issue_x(blk, eng=None):
                off, n = BOFF[blk], BLOCKS[blk]
                xt = xp.tile([KP, NCH * n], fp8, tag="xt", name=f"xt_{blk}")
                (eng or nc.sync).dma_start(
                    out=xt, in_=xsc[:, NCH * off:NCH * (off + n)]
                )
                xtiles[blk] = (xt, 0)

            def wslice(j, m):
                if j < NJ_HEAD:
                    return htw[:, (j * NM + m) * 128:(j * NM + m + 1) * 128]
                jj = j - NJ_HEAD
                return wt[:, (jj * NM + m) * 128:(jj * NM + m + 1) * 128]

            # Head DMAs fan out over three sequencers in parallel (a single
            # sequencer takes ~0.6-1.7 us per DMA issue): the fused
            # W(j0,j1)+X0 DMA via SP (fastest first-issue), W(j2..j4) via
            # gpsimd (needed ~1 us later), then X1..X3 queue behind on SP.
            nc.sync.dma_start(out=ht, in_=hd[:, :])
            nc.gpsimd.dma_start(
                out=wt, in_=wr[:, NJ_HEAD * NM * 128:]
            )
            xtiles[0] = (ht, HEAD_WB)
            issue_x(1); issue_x(2); issue_x(3)

            for blk in range(NBLK):
                off, n = BOFF[blk], BLOCKS[blk]
                if blk + 3 < NBLK:
                    issue_x(blk + 3)
                xt, xb = xtiles.pop(blk)
                psl = None
                if blk == NBLK - 1:
                    # Final block: all M-tiles share one PSUM tile so a
                    # single CAST + single DMA drain the kernel.
                    psl = pp.tile([128, NM * n], f32, tag="ps", name="ps_last")
                    pss = [psl[:, m * n:(m + 1) * n] for m in range(NM)]
                else:
                    pss = [
                        pp.tile([128, n], f32, tag="ps", name=f"ps_{blk}_{m}")
                        for m in range(NM)
                    ]
                ot = op.tile([128, NM * n], bf16, tag="ot", name=f"ot_{blk}")

                def evac(m, n=n, pss=pss, ot=ot, psl=psl):
                    if psl is not None:
                        if m == NM - 1:
                            nc.vector.tensor_copy(out=ot, in_=psl)
                        return
                    nc.vector.tensor_copy(out=ot[:, m * n:(m + 1) * n], in_=pss[m])

                if blk < N_JOUTER:
                    # j-outer: each arriving W(j) piece feeds all 3 M-tiles.
                    for j in range(NCH):
                        for m in range(NM):
                            nc.tensor.matmul(
                                pss[m],
                                wslice(j, m),
                                xt[:, xb + j * n:xb + (j + 1) * n],
                                start=(j == 0),
                                stop=(j == NCH - 1),
                                skip_group_check=True,
                            )
                    for m in range(NM):
                        evac(m)
                else:
                    # m-outer: M-tiles complete one after another, so PSUM
                    # evacuation staggers across the block.
                    for m in range(NM):
                        for j in range(NCH):
                            nc.tensor.matmul(
                                pss[m],
                                wslice(j, m),
                                xt[:, xb + j * n:xb + (j + 1) * n],
                                start=(j == 0),
                                stop=(j == NCH - 1),
                                skip_group_check=(psl is not None),
                            )
                        evac(m)
                if blk == NBLK - 2:
                    # Split this block's output so SP is free to issue the
                    # final block's DMA immediately: m0,m1 via SP now, m2
                    # via Act (which idles at its wait until the CAST lands).
                    nc.sync.dma_start(
                        out=osc[:, NM * off:NM * off + 2 * n], in_=ot[:, :2 * n]
                    )
                    nc.scalar.dma_start(
                        out=osc[:, NM * off + 2 * n:NM * (off + n)],
                        in_=ot[:, 2 * n:],
                    )
                else:
                    nc.sync.dma_start(
                        out=osc[:, NM * off:NM * (off + n)], in_=ot
                    )
    nc.compile()   # bacc passes: split multi-waits (HW allows 1 wait/inst), DCE, reg alloc
    return nc


def kernel(x: np.ndarray, templates: np.ndarray, onset_delays: np.ndarray) -> np.ndarray:
    global LAST_RESULT
    x = np.ascontiguousarray(x, dtype=np.float32)
    templates = np.asarray(templates, dtype=np.float32)
    onset_delays = np.asarray(onset_delays, dtype=np.float32)

    W = _build_weights(templates, onset_delays)
    Xsc = _build_xsc(x)                                   # (NCORES, KP, NCH*CTOT)

    # Fused head DMA payload: W(j0,j1) raw bf16 bytes + X block-0 columns.
    Wb = np.ascontiguousarray(W[:, :NJ_HEAD]).reshape(KP, -1).view(FP8)
    assert Wb.shape[1] == HEAD_WB
    Hd = np.concatenate(
        [np.broadcast_to(Wb, (NCORES, KP, HEAD_WB)), Xsc[:, :, :HEAD_XB]], axis=2
    )
    Hd = np.ascontiguousarray(Hd)

    nc = _build_program()
    in_maps = [{"xsc": Xsc[c], "w": W, "hd": Hd[c]} for c in range(NCORES)]
    res = run_bass_kernel_spmd(nc, in_maps, core_ids=list(range(NCORES)))
    LAST_RESULT = res

    osc = np.stack([r["osc"] for r in res.results], axis=0)   # (NCORES,128,NM*CTOT)
    osc = osc.astype(np.float32)
    O = np.empty((NCORES, 128, NM, CTOT), dtype=np.float32)
    for off, n in zip(BOFF, BLOCKS):
        O[:, :, :, off:off + n] = (
            osc[:, :, NM * off:NM * (off + n)].reshape(NCORES, 128, NM, n)
        )
    o = O.reshape(NCORES, 8, E, NM, BPC, NCOLB)           # c, dd, e, m, b, n
    o = o.transpose(0, 4, 5, 3, 1, 2)                      # c, b, n, m, dd, e
    o = np.ascontiguousarray(o).reshape(B, NCOLB * Q, E)[:, :S, :]
    o = np.ascontiguousarray(o)
    o[:, S - 1, :] = 0.0                                   # reference zero-pads last column
    return o


# revision 28
# speedup vs baseline: 1.0049x; 1.0049x over previous
"""Trainium2 Bass kernel for nn_EventTemplateBank (batched 1-D template-bank conv).

Math: score[b,t,e] = sum_{f,l} delayed[e,f,l] * x[b, t+40-l, f] / (L*F),
with delayed = delay-shifted templates (zero fill) and x zero-padded.

Device formulation (per core, data-parallel over batch):
  Feature-packed Toeplitz: contraction partitions hold (feature, tap-chunk)
  pairs, K = 6*21 = 126, so one matmul contracts all 6 features over a
  21-tap window. Each rhs column covers Q=24 output positions; the 103-tap
  span (24+79) is accumulated over NCH=5 chunks. Outputs (d in [0,24),
  e in [0,16)) form NM=3 M-tiles of 128.
    X[(f,c), j, col=n] = x[b, 24n + 21j + c - 39, f]      (fp8 e3m4, host im2col)
    W[(f,c), j, m, (dd,e)] = delayed[e, f, 8m+dd+79-21j-c] / 480   (bf16)
    ps[m][(dd,e), n] += sum_j W[:, j, m].T @ X[:, j, n-block]
  163,920 column-passes/core = 68.3 us PE floor at 2.4 GHz.
  X moves as fp8e3 (e3m4: 4 mantissa bits; host-sim rel err 1.38e-2 < 2e-2)
  against bf16 stationary W (mixed-dtype matmul, 1 cycle/row), halving X DMA:
  6.9 MB X + 8.4 MB out + 0.5 MB W ~= 15.8 MB/core ~= 44 us at 360 GB/s --
  DMA is no longer a co-bottleneck with the PE.
  A warmup chain of dummy matmuls on a memset tile runs during the DMA head
  so the PE p-state ramp (~3 us at 1.2 GHz) completes before real work.
  PSUM f32 -> bf16 evac; host upcasts and re-permutes (t = 24n + 8m + dd).
"""

import numpy as np
import ml_dtypes

import concourse.mybir as mybir
from concourse import bacc
from concourse.bass_utils import run_bass_kernel_spmd
from concourse.tile import TileContext

BF16 = ml_dtypes.bfloat16
FP8 = ml_dtypes.float8_e3m4

# Problem shapes (hardcoded per contract)
B, S, F = 64, 32768, 6
E, L = 16, 80
MAX_DELAY = 10

NCORES = 8
BPC = B // NCORES          # batches per core
Q = 24                     # output positions per rhs column
C = 21                     # taps per feature per chunk
NCH = 5                    # accumulation chunks (cover 24+79 = 103 <= 105 taps)
KP = F * C                 # 126 contraction partitions
NM = 3                     # M-tiles: (dd in [0,8)) x (e in [0,16)) per tile
PADL = 39                  # chunk sample index = 24n + 21j + c - 39
NCOLB = (S + Q - 1) // Q   # 1366 columns per batch
CTOT = BPC * NCOLB         # 10928 columns per core
# Lead blocks run j-outer, gated on per-j W pieces; block 0 (128 cols) rides
# in ONE fused DMA together with W(j0,j1) so a single DMA-latency chain
# unlocks the first six matmul groups. 512-col steady state; shrinking tail
# blocks keep the final evac+output-DMA drain chain short.
BLOCKS = [128, 256] + [512] * 19 + [432, 368, 16]
assert sum(BLOCKS) == CTOT
NBLK = len(BLOCKS)
BOFF = [sum(BLOCKS[:i]) for i in range(NBLK)]
N_JOUTER = 2               # leading blocks run j-outer (gate on per-j W pieces)
NJ_HEAD = 2                # W(j0,j1) rides the fused head DMA
HEAD_WB = NJ_HEAD * NM * 128 * 2            # W bytes in the fused head DMA
HEAD_XB = NCH * BLOCKS[0]                   # X block-0 bytes in the head DMA
NWARM = 34                 # warmup fills the whole pre-data window (~3.6 us)

LAST_RESULT = None         # BassKernelResults of the most recent run (for profiling)


def _build_weights(templates: np.ndarray, onset_delays: np.ndarray) -> np.ndarray:
    """W[(f,c), j, m, 16dd+e] = delayed[e, f, 8m+dd+79-21j-c] / (L*F)."""
    d = np.round(np.clip(onset_delays, -MAX_DELAY, MAX_DELAY)).astype(np.int64)
    idx = np.arange(L)
    src = idx[None, None, :] - d[:, :, None]                 # (E,F,L)
    valid = (src >= 0) & (src < L)
    delayed = np.take_along_axis(templates, np.clip(src, 0, L - 1), axis=2)
    delayed = np.where(valid, delayed, 0.0).astype(np.float32) / float(L * F)

    f_i = np.arange(F)[:, None, None, None, None]
    c_i = np.arange(C)[None, :, None, None, None]
    j_i = np.arange(NCH)[None, None, :, None, None]
    dd_i = np.arange(8)[None, None, None, :, None]
    m_i = np.arange(NM)[None, None, None, None, :]
    l = (8 * m_i + dd_i) + 79 - 21 * j_i - c_i               # (F,C,NCH,8,NM)
    ok = (l >= 0) & (l < L)
    g = delayed[:, f_i, np.clip(l, 0, L - 1)]                # (E,F,C,NCH,8,NM)
    g = np.where(ok[None], g, 0.0)
    # -> [(f,c), j, m, dd, e]
    W = g.transpose(1, 2, 3, 5, 4, 0).reshape(KP, NCH, NM, 8 * E)
    return np.ascontiguousarray(W).astype(BF16)


def _build_xsc(x: np.ndarray) -> np.ndarray:
    """Xsc[core, (f,c), :] = block-major concat of [NCH, n_blk] chunk rows:
    chunk j of column col = 1366*b_local + n reads x[b, 24n + 21j + c - 39, f].
    Quantized to fp8 e3m4 once on the raw samples (all replicas identical)."""
    need = Q * (NCOLB - 1) + 21 * (NCH - 1) + C
    xpad = np.zeros((B, PADL + need, F), dtype=np.float32)
    xpad[:, PADL:PADL + S, :] = x
    xpad8 = xpad.astype(FP8)
    sb, st, sf = xpad8.strides
    # V[b, (f,c), j, n] = xpad8[b, 24n + 21j + c, f]
    V = np.lib.stride_tricks.as_strided(
        xpad8, shape=(B, F, C, NCH, NCOLB), strides=(sb, sf, st, 21 * st, Q * st)
    )
    V8 = np.ascontiguousarray(V).reshape(B, KP, NCH, NCOLB)
    Xc = np.empty((NCORES, KP, NCH, CTOT), dtype=FP8)
    for b in range(B):
        core, i = divmod(b, BPC)
        Xc[core, :, :, i * NCOLB:(i + 1) * NCOLB] = V8[b]
    out = np.empty((NCORES, KP, NCH * CTOT), dtype=FP8)
    for off, n in zip(BOFF, BLOCKS):
        out[:, :, NCH * off:NCH * (off + n)] = (
            Xc[:, :, :, off:off + n].reshape(NCORES, KP, NCH * n)
        )
    return np.ascontiguousarray(out)


def _build_program():
    f32 = mybir.dt.float32
    bf16 = mybir.dt.bfloat16
    fp8 = mybir.dt.float8e3
    nc = bacc.Bacc("TRN2", target_bir_lowering=False, debug=False)
    xsc = nc.dram_tensor("xsc", [KP, NCH * CTOT], fp8, kind="ExternalInput")
    # Fused head: W(j0,j1) raw bf16 bytes followed by X block-0, one DMA.
    hd = nc.dram_tensor("hd", [KP, HEAD_WB + HEAD_XB], fp8, kind="ExternalInput")
    w = nc.dram_tensor("w", [KP, NCH, NM, 128], bf16, kind="ExternalInput")
    osc = nc.dram_tensor("osc", [128, NM * CTOT], bf16, kind="ExternalOutput")

    with TileContext(nc) as tc:
        with (
            tc.tile_pool(name="wp", bufs=1) as wp,
            tc.tile_pool(name="xp", bufs=6) as xp,
            tc.tile_pool(name="pp", bufs=7, space="PSUM") as pp,
            tc.tile_pool(name="wu", bufs=1) as wup,
            tc.tile_pool(name="wq", bufs=1, space="PSUM") as wqp,
            tc.tile_pool(name="op", bufs=6) as op,
        ):
            # --- PE warmup: dummy matmuls on a zeroed tile ramp the tensor
            # engine's p-state to full clock while the first DMAs land.
            wlhs = wup.tile([128, 128], bf16)
            nc.gpsimd.memset(wlhs, 0)
            wps = wqp.tile([128, 128], f32)
            for _ in range(NWARM):
                nc.tensor.matmul(
                    wps, wlhs, wlhs, start=True, stop=True, skip_group_check=True
                )

            ht = wp.tile([KP, HEAD_WB + HEAD_XB], fp8)   # fused W(j0,j1) + X0
            htw = ht[:, :HEAD_WB].bitcast(bf16)          # [(f,c), 768 bf16]
            wt = wp.tile([KP, (NCH - NJ_HEAD) * NM * 128], bf16)  # W(j2..j4)
            wr = w.rearrange("k j m n -> k (j m n)")
            xtiles = {}

            def issue_x(blk, eng=None):
                off, n = BOFF[blk], BLOCKS[blk]
                xt = xp.tile([KP, NCH * n], fp8, tag="xt", name=f"xt_{blk}")
                (eng or nc.sync).dma_start(
                    out=xt, in_=xsc[:, NCH * off:NCH * (off + n)]
                )
                xtiles[blk] = (xt, 0)

            def wslice(j, m):
                if j < NJ_HEAD:
                    return htw[:, (j * NM + m) * 128:(j * NM + m + 1) * 128]
                jj = j - NJ_HEAD
                return wt[:, (jj * NM + m) * 128:(jj * NM + m + 1) * 128]

            # Head DMAs fan out over three sequencers in parallel (a single
            # sequencer takes ~0.6-1.7 us per DMA issue): the fused
            # W(j0,j1)+X0 DMA via SP (fastest first-issue), W(j2..j4) via
            # gpsimd (needed ~1 us later), then X1..X3 queue behind on SP.
            nc.sync.dma_start(out=ht, in_=hd[:, :])
            nc.gpsimd.dma_start(
                out=wt, in_=wr[:, NJ_HEAD * NM * 128:]
            )
            xtiles[0] = (ht, HEAD_WB)
            issue_x(1); issue_x(2); issue_x(3)

            for blk in range(NBLK):
                off, n = BOFF[blk], BLOCKS[blk]
                if blk + 3 < NBLK:
                    issue_x(blk + 3)
                xt, xb = xtiles.pop(blk)
                psl = None
                if blk == NBLK - 1:
                    # Final block: all M-tiles share one PSUM tile so a
                    # single CAST + single DMA drain the kernel.
                    psl = pp.tile([128, NM * n], f32, tag="ps", name="ps_last")
                    pss = [psl[:, m * n:(m + 1) * n] for m in range(NM)]
                else:
                    pss = [
                        pp.tile([128, n], f32, tag="ps", name=f"ps_{blk}_{m}")
                        for m in range(NM)
                    ]
                ot = op.tile([128, NM * n], bf16, tag="ot", name=f"ot_{blk}")

                def evac(m, n=n, pss=pss, ot=ot, psl=psl):
                    if psl is not None:
                        if m == NM - 1:
                            nc.vector.tensor_copy(out=ot, in_=psl)
                        return
                    nc.vector.tensor_copy(out=ot[:, m * n:(m + 1) * n], in_=pss[m])

                if blk < N_JOUTER:
                    # j-outer: each arriving W(j) piece feeds all 3 M-tiles.
                    for j in range(NCH):
                        for m in range(NM):
                            nc.tensor.matmul(
                                pss[m],
                                wslice(j, m),
                                xt[:, xb + j * n:xb + (j + 1) * n],
                                start=(j == 0),
                                stop=(j == NCH - 1),
                                skip_group_check=True,
                            )
                    for m in range(NM):
                        evac(m)
                else:
                    # m-outer: M-tiles complete one after another, so PSUM
                    # evacuation staggers across the block.
                    for m in range(NM):
                        for j in range(NCH):
                            nc.tensor.matmul(
                                pss[m],
                                wslice(j, m),
                                xt[:, xb + j * n:xb + (j + 1) * n],
                                start=(j == 0),
                                stop=(j == NCH - 1),
                                skip_group_check=(psl is not None),
                            )
                        evac(m)
                if blk == NBLK - 2:
                    # Split this block's output so SP is free to issue the
                    # final block's DMA immediately: m0,m1 via SP now, m2
                    # via Act (which idles at its wait until the CAST lands).
                    nc.sync.dma_start(
                        out=osc[:, NM * off:NM * off + 2 * n], in_=ot[:, :2 * n]
                    )
                    nc.scalar.dma_start(
                        out=osc[:, NM * off + 2 * n:NM * (off + n)],
                        in_=ot[:, 2 * n:],
                    )
                else:
                    nc.sync.dma_start(
                        out=osc[:, NM * off:NM * (off + n)], in_=ot
                    )
    nc.compile()   # bacc passes: split multi-waits (HW allows 1 wait/inst), DCE, reg alloc
    return nc


def kernel(x: np.ndarray, templates: np.ndarray, onset_delays: np.ndarray) -> np.ndarray:
    global LAST_RESULT
    x = np.ascontiguousarray(x, dtype=np.float32)
    templates = np.asarray(templates, dtype=np.float32)
    onset_delays = np.asarray(onset_delays, dtype=np.float32)

    W = _build_weights(templates, onset_delays)
    Xsc = _build_xsc(x)                                   # (NCORES, KP, NCH*CTOT)

    # Fused head DMA payload: W(j0,j1) raw bf16 bytes + X block-0 columns.
    Wb = np.ascontiguousarray(W[:, :NJ_HEAD]).reshape(KP, -1).view(FP8)
    assert Wb.shape[1] == HEAD_WB
    Hd = np.concatenate(
        [np.broadcast_to(Wb, (NCORES, KP, HEAD_WB)), Xsc[:, :, :HEAD_XB]], axis=2
    )
    Hd = np.ascontiguousarray(Hd)

    nc = _build_program()
    in_maps = [{"xsc": Xsc[c], "w": W, "hd": Hd[c]} for c in range(NCORES)]
    res = run_bass_kernel_spmd(nc, in_maps, core_ids=list(range(NCORES)))
    LAST_RESULT = res

    osc = np.stack([r["osc"] for r in res.results], axis=0)   # (NCORES,128,NM*CTOT)
    osc = osc.astype(np.float32)
    O = np.empty((NCORES, 128, NM, CTOT), dtype=np.float32)
    for off, n in zip(BOFF, BLOCKS):
        O[:, :, :, off:off + n] = (
            osc[:, :, NM * off:NM * (off + n)].reshape(NCORES, 128, NM, n)
        )
    o = O.reshape(NCORES, 8, E, NM, BPC, NCOLB)           # c, dd, e, m, b, n
    o = o.transpose(0, 4, 5, 3, 1, 2)                      # c, b, n, m, dd, e
    o = np.ascontiguousarray(o).reshape(B, NCOLB * Q, E)[:, :S, :]
    o = np.ascontiguousarray(o)
    o[:, S - 1, :] = 0.0                                   # reference zero-pads last column
    return o


# revision 31
# speedup vs baseline: 1.0082x; 1.0032x over previous
"""Trainium2 Bass kernel for nn_EventTemplateBank (batched 1-D template-bank conv).

Math: score[b,t,e] = sum_{f,l} delayed[e,f,l] * x[b, t+40-l, f] / (L*F),
with delayed = delay-shifted templates (zero fill) and x zero-padded.

Device formulation (per core, data-parallel over batch):
  Feature-packed Toeplitz: contraction partitions hold (feature, tap-chunk)
  pairs, K = 6*21 = 126, so one matmul contracts all 6 features over a
  21-tap window. Each rhs column covers Q=24 output positions; the 103-tap
  span (24+79) is accumulated over NCH=5 chunks. Outputs (d in [0,24),
  e in [0,16)) form NM=3 M-tiles of 128.
    X[(f,c), j, col=n] = x[b, 24n + 21j + c - 39, f]      (fp8 e3m4, host im2col)
    W[(f,c), j, m, (dd,e)] = delayed[e, f, 8m+dd+79-21j-c] / 480   (bf16)
    ps[m][(dd,e), n] += sum_j W[:, j, m].T @ X[:, j, n-block]
  163,920 column-passes/core = 68.3 us PE floor at 2.4 GHz.
  X moves as fp8e3 (e3m4: 4 mantissa bits; host-sim rel err 1.38e-2 < 2e-2)
  against bf16 stationary W (mixed-dtype matmul, 1 cycle/row), halving X DMA:
  6.9 MB X + 8.4 MB out + 0.5 MB W ~= 15.8 MB/core ~= 44 us at 360 GB/s --
  DMA is no longer a co-bottleneck with the PE.
  A warmup chain of dummy matmuls on a memset tile runs during the DMA head
  so the PE p-state ramp (~3 us at 1.2 GHz) completes before real work.
  PSUM f32 -> bf16 evac; host upcasts and re-permutes (t = 24n + 8m + dd).
"""

import numpy as np
import ml_dtypes

import concourse.mybir as mybir
from concourse import bacc
from concourse.bass_utils import run_bass_kernel_spmd
from concourse.tile import TileContext

BF16 = ml_dtypes.bfloat16
FP8 = ml_dtypes.float8_e3m4

# Problem shapes (hardcoded per contract)
B, S, F = 64, 32768, 6
E, L = 16, 80
MAX_DELAY = 10

NCORES = 8
BPC = B // NCORES          # batches per core
Q = 24                     # output positions per rhs column
C = 21                     # taps per feature per chunk
NCH = 5                    # accumulation chunks (cover 24+79 = 103 <= 105 taps)
KP = F * C                 # 126 contraction partitions
NM = 3                     # M-tiles: (dd in [0,8)) x (e in [0,16)) per tile
PADL = 39                  # chunk sample index = 24n + 21j + c - 39
NCOLB = (S + Q - 1) // Q   # 1366 columns per batch
CTOT = BPC * NCOLB         # 10928 columns per core
# Lead blocks run j-outer, gated on per-j W pieces; block 0 (128 cols) rides
# in ONE fused DMA together with W(j0,j1) so a single DMA-latency chain
# unlocks the first six matmul groups. 512-col steady state; shrinking tail
# blocks keep the final evac+output-DMA drain chain short.
BLOCKS = [128, 256] + [512] * 19 + [432, 368, 16]
assert sum(BLOCKS) == CTOT
NBLK = len(BLOCKS)
BOFF = [sum(BLOCKS[:i]) for i in range(NBLK)]
N_JOUTER = 2               # leading blocks run j-outer (gate on per-j W pieces)
NJ_HEAD = 2                # W(j0,j1) rides the fused head DMA
HEAD_WB = NJ_HEAD * NM * 128 * 2            # W bytes in the fused head DMA
HEAD_XB = NCH * BLOCKS[0]                   # X block-0 bytes in the head DMA
NWARM = 31                 # warmup fills the pre-data window (~3.3 us)
NWARM_TAPER = 6            # last warmups use 64-col streams: finer handoff

LAST_RESULT = None         # BassKernelResults of the most recent run (for profiling)


def _build_weights(templates: np.ndarray, onset_delays: np.ndarray) -> np.ndarray:
    """W[(f,c), j, m, 16dd+e] = delayed[e, f, 8m+dd+79-21j-c] / (L*F)."""
    d = np.round(np.clip(onset_delays, -MAX_DELAY, MAX_DELAY)).astype(np.int64)
    idx = np.arange(L)
    src = idx[None, None, :] - d[:, :, None]                 # (E,F,L)
    valid = (src >= 0) & (src < L)
    delayed = np.take_along_axis(templates, np.clip(src, 0, L - 1), axis=2)
    delayed = np.where(valid, delayed, 0.0).astype(np.float32) / float(L * F)

    f_i = np.arange(F)[:, None, None, None, None]
    c_i = np.arange(C)[None, :, None, None, None]
    j_i = np.arange(NCH)[None, None, :, None, None]
    dd_i = np.arange(8)[None, None, None, :, None]
    m_i = np.arange(NM)[None, None, None, None, :]
    l = (8 * m_i + dd_i) + 79 - 21 * j_i - c_i               # (F,C,NCH,8,NM)
    ok = (l >= 0) & (l < L)
    g = delayed[:, f_i, np.clip(l, 0, L - 1)]                # (E,F,C,NCH,8,NM)
    g = np.where(ok[None], g, 0.0)
    # -> [(f,c), j, m, dd, e]
    W = g.transpose(1, 2, 3, 5, 4, 0).reshape(KP, NCH, NM, 8 * E)
    return np.ascontiguousarray(W).astype(BF16)


def _build_xsc(x: np.ndarray) -> np.ndarray:
    """Xsc[core, (f,c), :] = block-major concat of [NCH, n_blk] chunk rows:
    chunk j of column col = 1366*b_local + n reads x[b, 24n + 21j + c - 39, f].
    Quantized to fp8 e3m4 once on the raw samples (all replicas identical)."""
    need = Q * (NCOLB - 1) + 21 * (NCH - 1) + C
    xpad = np.zeros((B, PADL + need, F), dtype=np.float32)
    xpad[:, PADL:PADL + S, :] = x
    xpad8 = xpad.astype(FP8)
    sb, st, sf = xpad8.strides
    # V[b, (f,c), j, n] = xpad8[b, 24n + 21j + c, f]
    V = np.lib.stride_tricks.as_strided(
        xpad8, shape=(B, F, C, NCH, NCOLB), strides=(sb, sf, st, 21 * st, Q * st)
    )
    V8 = np.ascontiguousarray(V).reshape(B, KP, NCH, NCOLB)
    Xc = np.empty((NCORES, KP, NCH, CTOT), dtype=FP8)
    for b in range(B):
        core, i = divmod(b, BPC)
        Xc[core, :, :, i * NCOLB:(i + 1) * NCOLB] = V8[b]
    out = np.empty((NCORES, KP, NCH * CTOT), dtype=FP8)
    for off, n in zip(BOFF, BLOCKS):
        out[:, :, NCH * off:NCH * (off + n)] = (
            Xc[:, :, :, off:off + n].reshape(NCORES, KP, NCH * n)
        )
    return np.ascontiguousarray(out)


def _build_program():
    f32 = mybir.dt.float32
    bf16 = mybir.dt.bfloat16
    fp8 = mybir.dt.float8e3
    nc = bacc.Bacc("TRN2", target_bir_lowering=False, debug=False)
    xsc = nc.dram_tensor("xsc", [KP, NCH * CTOT], fp8, kind="ExternalInput")
    # Fused head: W(j0,j1) raw bf16 bytes followed by X block-0, one DMA.
    hd = nc.dram_tensor("hd", [KP, HEAD_WB + HEAD_XB], fp8, kind="ExternalInput")
    w = nc.dram_tensor("w", [KP, NCH, NM, 128], bf16, kind="ExternalInput")
    osc = nc.dram_tensor("osc", [128, NM * CTOT], bf16, kind="ExternalOutput")

    with TileContext(nc) as tc:
        with (
            tc.tile_pool(name="wp", bufs=1) as wp,
            tc.tile_pool(name="xp", bufs=6) as xp,
            tc.tile_pool(name="pp", bufs=7, space="PSUM") as pp,
            tc.tile_pool(name="wu", bufs=1) as wup,
            tc.tile_pool(name="wq", bufs=1, space="PSUM") as wqp,
            tc.tile_pool(name="op", bufs=6) as op,
        ):
            # --- PE warmup: dummy matmuls on a zeroed tile ramp the tensor
            # engine's p-state to full clock while the first DMAs land.
            wlhs = wup.tile([128, 128], bf16)
            nc.gpsimd.memset(wlhs, 0)
            wps = wqp.tile([128, 128], f32)
            for i in range(NWARM):
                taper = i >= NWARM - NWARM_TAPER
                nc.tensor.matmul(
                    wps[:, :64] if taper else wps,
                    wlhs,
                    wlhs[:, :64] if taper else wlhs,
                    start=True, stop=True, skip_group_check=True,
                )

            ht = wp.tile([KP, HEAD_WB + HEAD_XB], fp8)   # fused W(j0,j1) + X0
            htw = ht[:, :HEAD_WB].bitcast(bf16)          # [(f,c), 768 bf16]
            wt = wp.tile([KP, (NCH - NJ_HEAD) * NM * 128], bf16)  # W(j2..j4)
            wr = w.rearrange("k j m n -> k (j m n)")
            xtiles = {}

            def issue_x(blk, eng=None):
                off, n = BOFF[blk], BLOCKS[blk]
                xt = xp.tile([KP, NCH * n], fp8, tag="xt", name=f"xt_{blk}")
                (eng or nc.sync).dma_start(
                    out=xt, in_=xsc[:, NCH * off:NCH * (off + n)]
                )
                xtiles[blk] = (xt, 0)

            def wslice(j, m):
                if j < NJ_HEAD:
                    return htw[:, (j * NM + m) * 128:(j * NM + m + 1) * 128]
                jj = j - NJ_HEAD
                return wt[:, (jj * NM + m) * 128:(jj * NM + m + 1) * 128]

            # Head DMAs fan out over three sequencers in parallel (a single
            # sequencer takes ~0.6-1.7 us per DMA issue): the fused
            # W(j0,j1)+X0 DMA via SP (fastest first-issue), W(j2..j4) via
            # gpsimd (needed ~1 us later), then X1..X3 queue behind on SP.
            nc.sync.dma_start(out=ht, in_=hd[:, :])
            nc.gpsimd.dma_start(
                out=wt, in_=wr[:, NJ_HEAD * NM * 128:]
            )
            xtiles[0] = (ht, HEAD_WB)
            issue_x(1); issue_x(2)

            for blk in range(NBLK):
                off, n = BOFF[blk], BLOCKS[blk]
                if blk + 3 < NBLK:
                    issue_x(blk + 3)
                xt, xb = xtiles.pop(blk)
                psl = None
                if blk == NBLK - 1:
                    # Final block: all M-tiles share one PSUM tile so a
                    # single CAST + single DMA drain the kernel.
                    psl = pp.tile([128, NM * n], f32, tag="ps", name="ps_last")
                    pss = [psl[:, m * n:(m + 1) * n] for m in range(NM)]
                else:
                    pss = [
                        pp.tile([128, n], f32, tag="ps", name=f"ps_{blk}_{m}")
                        for m in range(NM)
                    ]
                ot = op.tile([128, NM * n], bf16, tag="ot", name=f"ot_{blk}")

                def evac(m, n=n, pss=pss, ot=ot, psl=psl):
                    if psl is not None:
                        if m == NM - 1:
                            nc.vector.tensor_copy(out=ot, in_=psl)
                        return
                    nc.vector.tensor_copy(out=ot[:, m * n:(m + 1) * n], in_=pss[m])

                if blk < N_JOUTER:
                    # j-outer: each arriving W(j) piece feeds all 3 M-tiles.
                    for j in range(NCH):
                        for m in range(NM):
                            nc.tensor.matmul(
                                pss[m],
                                wslice(j, m),
                                xt[:, xb + j * n:xb + (j + 1) * n],
                                start=(j == 0),
                                stop=(j == NCH - 1),
                                skip_group_check=True,
                            )
                    for m in range(NM):
                        evac(m)
                else:
                    # m-outer: M-tiles complete one after another, so PSUM
                    # evacuation staggers across the block.
                    for m in range(NM):
                        for j in range(NCH):
                            nc.tensor.matmul(
                                pss[m],
                                wslice(j, m),
                                xt[:, xb + j * n:xb + (j + 1) * n],
                                start=(j == 0),
                                stop=(j == NCH - 1),
                                skip_group_check=(psl is not None),
                            )
                        evac(m)
                if blk == NBLK - 2:
                    # Split this block's output so SP is free to issue the
                    # final block's DMA immediately: m0,m1 via SP now, m2
                    # via Act (which idles at its wait until the CAST lands).
                    nc.sync.dma_start(
                        out=osc[:, NM * off:NM * off + 2 * n], in_=ot[:, :2 * n]
                    )
                    nc.scalar.dma_start(
                        out=osc[:, NM * off + 2 * n:NM * (off + n)],
                        in_=ot[:, 2 * n:],
                    )
                else:
                    nc.sync.dma_start(
                        out=osc[:, NM * off:NM * (off + n)], in_=ot
                    )
    nc.compile()   # bacc passes: split multi-waits (HW allows 1 wait/inst), DCE, reg alloc
    return nc


def kernel(x: np.ndarray, templates: np.ndarray, onset_delays: np.ndarray) -> np.ndarray:
    global LAST_RESULT
    x = np.ascontiguousarray(x, dtype=np.float32)
    templates = np.asarray(templates, dtype=np.float32)
    onset_delays = np.asarray(onset_delays, dtype=np.float32)

    W = _build_weights(templates, onset_delays)
    Xsc = _build_xsc(x)                                   # (NCORES, KP, NCH*CTOT)

    # Fused head DMA payload: W(j0,j1) raw bf16 bytes + X block-0 columns.
    Wb = np.ascontiguousarray(W[:, :NJ_HEAD]).reshape(KP, -1).view(FP8)
    assert Wb.shape[1] == HEAD_WB
    Hd = np.concatenate(
        [np.broadcast_to(Wb, (NCORES, KP, HEAD_WB)), Xsc[:, :, :HEAD_XB]], axis=2
    )
    Hd = np.ascontiguousarray(Hd)

    nc = _build_program()
    in_maps = [{"xsc": Xsc[c], "w": W, "hd": Hd[c]} for c in range(NCORES)]
    res = run_bass_kernel_spmd(nc, in_maps, core_ids=list(range(NCORES)))
    LAST_RESULT = res

    osc = np.stack([r["osc"] for r in res.results], axis=0)   # (NCORES,128,NM*CTOT)
    osc = osc.astype(np.float32)
    O = np.empty((NCORES, 128, NM, CTOT), dtype=np.float32)
    for off, n in zip(BOFF, BLOCKS):
        O[:, :, :, off:off + n] = (
            osc[:, :, NM * off:NM * (off + n)].reshape(NCORES, 128, NM, n)
        )
    o = O.reshape(NCORES, 8, E, NM, BPC, NCOLB)           # c, dd, e, m, b, n
    o = o.transpose(0, 4, 5, 3, 1, 2)                      # c, b, n, m, dd, e
    o = np.ascontiguousarray(o).reshape(B, NCOLB * Q, E)[:, :S, :]
    o = np.ascontiguousarray(o)
    o[:, S - 1, :] = 0.0                                   # reference zero-pads last column
    return o
